# revision 2
# baseline (speedup 1.0000x reference)
"""Single-head attention on Trainium2: out = softmax(x Wq (x Wk)^T / sqrt(64)) (x Wv).

Full inputs: x [8, 2048, 512], Wq/Wk/Wv [512, 64]. Data-parallel over batch:
core b computes batch element b. Inside each core:
  - transpose x -> x^T (TensorE transposes, fp32)
  - projections q^T, k^T, v^T = W^T x^T  (float32r matmuls)
  - v^T transposed back to natural v, with an appended ones-column
  - for each 128-row k-tile: S^T tile = k_tile q^T (PSUM), exp on ScalarE
    (scale=1/8 folded in), then PV accumulation out^T += [v|1]^T P^T, whose
    last row accumulates the softmax denominators.
  - divide by denominators after transposing back to natural layout; DMA out.
"""

import numpy as np

B, S, E, D = 8, 2048, 512, 64
NCORES = 8
NT = S // 128  # 16 s-tiles
NE = E // 128  # 4 e-chunks
NQ = S // 512  # 4 q-chunks of 512
SCALE = 1.0 / float(np.sqrt(D))

_CACHE = {}


def _build():
    import concourse.bass as bass
    import concourse.tile as tile
    from concourse import bacc, mybir
    from concourse.masks import make_identity

    f32 = mybir.dt.float32
    f32r = mybir.dt.float32r
    AF = mybir.ActivationFunctionType

    nc = bacc.Bacc("TRN2", target_bir_lowering=False, debug=False,
                   num_devices=NCORES)

    x_d = nc.dram_tensor("x", [S, E], f32, kind="ExternalInput").ap()
    wq_d = nc.dram_tensor("Wq", [E, D], f32r, kind="ExternalInput").ap()
    wk_d = nc.dram_tensor("Wk", [E, D], f32r, kind="ExternalInput").ap()
    wv_d = nc.dram_tensor("Wv", [E, D], f32r, kind="ExternalInput").ap()
    out_d = nc.dram_tensor("out", [S, D], f32, kind="ExternalOutput").ap()

    with tile.TileContext(nc) as tc:
        with (
            tc.tile_pool(name="persist", bufs=1) as pp,
            tc.tile_pool(name="ptp", bufs=2) as ptp,
            tc.tile_pool(name="small", bufs=4) as sp,
            tc.tile_pool(name="ps", bufs=1, space="PSUM") as ps,
        ):
            ident = pp.tile([128, 128], f32)
            make_identity(nc, ident[:])

            # ---- loads ----
            wq_s = pp.tile([128, NE, D], f32r)
            nc.sync.dma_start(wq_s[:], wq_d.rearrange("(a p) d -> p a d", p=128))
            wk_s = pp.tile([128, NE, D], f32r)
            nc.sync.dma_start(wk_s[:], wk_d.rearrange("(a p) d -> p a d", p=128))
            wv_s = pp.tile([128, NE, D], f32r)
            nc.sync.dma_start(wv_s[:], wv_d.rearrange("(a p) d -> p a d", p=128))
            x_s = pp.tile([128, NT, E], f32)
            nc.sync.dma_start(x_s[:], x_d.rearrange("(t p) e -> p t e", p=128))

            # ---- phase 2: x^T via TensorE transposes ----
            xT = pp.tile([128, NE, S], f32r)  # [e_part, ec, s]
            for ec in range(NE):
                tag = "a" if ec % 2 == 0 else "b"
                pst = ps.tile([128, S], f32, tag=tag, name=f"xtp{ec}")
                for st in range(NT):
                    nc.tensor.transpose(
                        pst[:, st * 128:(st + 1) * 128],
                        x_s[:, st, ec * 128:(ec + 1) * 128],
                        ident[:],
                    )
                if ec % 2 == 0:
                    nc.scalar.copy(xT[:, ec, :], pst[:])
                else:
                    nc.vector.tensor_copy(xT[:, ec, :], pst[:])

            # ---- phase 3: projections q^T,k^T [64, S] and v^T [64, S] ----
            qkT = pp.tile([64, 2, S], f32r)
            vT = pp.tile([64, S], f32)
            for i, w_s in enumerate([wq_s, wk_s, wv_s]):
                tag = ["a", "b", "a"][i]
                pj = ps.tile([64, S], f32, tag=tag, name=f"proj{i}")
                for sc in range(NQ):
                    for ec in range(NE):
                        nc.tensor.matmul(
                            pj[:, sc * 512:(sc + 1) * 512],
                            w_s[:, ec, :],
                            xT[:, ec, sc * 512:(sc + 1) * 512],
                            start=(ec == 0),
                            stop=(ec == NE - 1),
                        )
                dst = qkT[:, i, :] if i < 2 else vT[:]
                if i % 2 == 0:
                    nc.vector.tensor_copy(dst, pj[:])
                else:
                    nc.scalar.copy(dst, pj[:])

            # ---- v natural [128, NT, 65] with ones column ----
            v_sb = pp.tile([128, NT, D + 1], f32r)
            nc.gpsimd.memset(v_sb[:, :, D:D + 1].bitcast(f32), 1.0)
            vnp = ps.tile([128, S], f32, tag="b", name="vnat")
            for st in range(NT):
                nc.tensor.transpose(
                    vnp[:, st * 128: st * 128 + D],
                    vT[:, st * 128:(st + 1) * 128],
                    ident[0:D, 0:D],
                )
            nc.vector.tensor_copy(
                v_sb[:, :, 0:D],
                vnp.rearrange("p (t c) -> p t c", c=128)[:, :, 0:D],
            )

            # ---- main loop over k-tiles ----
            outT = ps.tile([D + 1, S], f32, tag="b", name="outT")
            for kt in range(NT):
                sT = ps.tile([128, S], f32, tag="a", name=f"sT{kt}")
                for qc in range(NQ):
                    nc.tensor.matmul(
                        sT[:, qc * 512:(qc + 1) * 512],
                        qkT[:, 1, kt * 128:(kt + 1) * 128],
                        qkT[:, 0, qc * 512:(qc + 1) * 512],
                        start=True,
                        stop=True,
                    )
                pT = ptp.tile([128, S], f32r, name="pT")
                nc.scalar.activation(pT[:], sT[:], AF.Exp, scale=SCALE)
                for qc in range(NQ):
                    nc.tensor.matmul(
                        outT[:, qc * 512:(qc + 1) * 512],
                        v_sb[:, kt, :],
                        pT[:, qc * 512:(qc + 1) * 512],
                        start=(kt == 0),
                        stop=(kt == NT - 1),
                        skip_group_check=True,
                    )

            # ---- tail: normalize + transpose back + store ----
            outT_sb = pp.tile([D + 1, S], f32)
            for qc in range(NQ):
                if qc % 2 == 0:
                    nc.scalar.copy(outT_sb[:, qc * 512:(qc + 1) * 512],
                                   outT[:, qc * 512:(qc + 1) * 512])
                else:
                    nc.vector.tensor_copy(outT_sb[:, qc * 512:(qc + 1) * 512],
                                          outT[:, qc * 512:(qc + 1) * 512])
            out_sb = pp.tile([128, NT, D], f32)
            for st in range(NT):
                tag = "a" if st % 2 == 0 else "b"
                nat = ps.tile([128, D + 1], f32, tag=tag, name=f"nat{st}")
                nc.tensor.transpose(
                    nat[:, 0:D + 1],
                    outT_sb[:, st * 128:(st + 1) * 128],
                    ident[0:D + 1, 0:D + 1],
                )
                lrec = sp.tile([128, 1], f32, name="lrec")
                nc.vector.reciprocal(lrec[:], nat[:, D:D + 1])
                nc.scalar.activation(out_sb[:, st, :], nat[:, 0:D],
                                     AF.Copy, scale=lrec[:])
            nc.sync.dma_start(out_d.rearrange("(t p) d -> p t d", p=128),
                              out_sb[:])

    nc.compile()
    return nc


def kernel(**inputs):
    from concourse.bass_utils import run_bass_kernel_spmd

    x = np.ascontiguousarray(np.asarray(inputs["x"], dtype=np.float32))
    wq = np.ascontiguousarray(np.asarray(inputs["Wq"], dtype=np.float32))
    wk = np.ascontiguousarray(np.asarray(inputs["Wk"], dtype=np.float32))
    wv = np.ascontiguousarray(np.asarray(inputs["Wv"], dtype=np.float32))

    if "nc" not in _CACHE:
        _CACHE["nc"] = _build()
    nc = _CACHE["nc"]

    in_maps = [
        {"x": np.ascontiguousarray(x[b]), "Wq": wq, "Wk": wk, "Wv": wv}
        for b in range(B)
    ]
    res = run_bass_kernel_spmd(nc, in_maps, core_ids=list(range(NCORES)))
    _CACHE["last_results"] = res
    out = np.stack([res.results[b]["out"] for b in range(B)], axis=0)
    return out
